# revision 26
# baseline (speedup 1.0000x reference)
"""Trainium2 Bass kernel for nn_ConvProjector (conv3x3 -> ReLU -> conv3x3 -> ReLU
-> adaptive-avg-pool upsample 32x32 -> 687x1024 -> 1x1 conv 256->24 + bias).

Strategy:
  * The adaptive pool (linear) and the 1x1 conv (linear) commute: apply the
    256->24 channel reduction at 32x32 resolution first.  The device then
    emits only the DISTINCT values of the output: the reduced tensor
    r = wr @ h2 at 32x32 (every output column is a pure replica of an input
    column since 1024 = 32*32, and every output row is either a replica or
    the mean of two adjacent rows).  The pool expansion to 687x1024 plus the
    +bias is exact fp32 gather arithmetic done while unsharding on the host.
  * Sharding: 8 cores, core k owns rows 4k..4k+3.  It computes h1 rows
    4k-1..4k+4 (conv halos) from x rows 4k-2..4k+5, h2/r rows 4k..4k+3,
    channel-complete.  No collectives.
  * conv1 bias+image-row masking is folded into the matmul accumulation
    (rank-1 update b1 x maskrow), so no mask tensor or vector ops.
  * All matmuls run in fp16 (inputs pre-rounded on host; PSUM stays fp32).
  * Weight DMA: per-(tap, mc) w1 tiles and per-kc x chunks are assigned to
    the three DMA-capable queues (sync/scalar/gpsimd) by a greedy scheduler
    in consumption order, so conv1 starts early and streams behind arrival.
"""
import sys

if '/opt/trn_rl_repo' not in sys.path:
    sys.path.insert(0, '/opt/trn_rl_repo')

import numpy as np

IN_C, MID_C, OUT_C = 576, 256, 24
H = W = 32
OUT_H, OUT_W = 687, 1024
NCORES = 8
P = 128
KC1 = 5           # ceil(576/128) input-channel chunks for conv1 (padded to 640)
KC2 = 2           # 256/128 chunks for conv2 / 1x1
MC = 2            # 256/128 output-channel chunks for conv1/conv2
WP = 34           # padded row width (1 zero col each side)
RX, R1, R2 = 8, 6, 4          # x rows / h1 rows / h2 (=r) rows per core
XBLK = RX * WP                # 272  per-kc x block
XSLACK = 16                   # rhs overrun slack
N1 = R1 * WP                  # 204  conv1 matmul N
H1BLK = R1 * WP               # 204  per-mc h1 block
H1SLACK = 16
N2 = R2 * WP                  # 136  conv2 matmul N
H2BLK = R2 * WP               # 136  h2 flat span per mc
BMW = 512                     # biasmask row: [0:N1] mask, [256:384] b1 mc0,
                              # [384:512] b1 mc1

_prog_cache = {}


def _build_program():
    import concourse.bass as bass
    import concourse.bacc as bacc
    import concourse.mybir as mybir
    from concourse.tile import TileContext

    f32 = mybir.dt.float32
    f16 = mybir.dt.float16
    nc = bacc.Bacc("TRN2", target_bir_lowering=False, debug=False,
                   num_devices=NCORES)

    xs_d = nc.dram_tensor("xs", [P, KC1 * XBLK + XSLACK], f16, kind="ExternalInput")
    w1_d = nc.dram_tensor("w1p", [P, 9 * KC1 * MC * P], f16, kind="ExternalInput")
    w2_d = nc.dram_tensor("w2p", [P, 9 * KC2 * MC * P], f16, kind="ExternalInput")
    wr_d = nc.dram_tensor("wrp", [P, KC2 * OUT_C], f16, kind="ExternalInput")
    bm_d = nc.dram_tensor("bmp", [2, BMW], f16, kind="ExternalInput")
    b2_d = nc.dram_tensor("b2p", [P, MC], f32, kind="ExternalInput")
    out_d = nc.dram_tensor("outb", [W, R2 * OUT_C], f32, kind="ExternalOutput")

    Relu = mybir.ActivationFunctionType.Relu
    Ident = mybir.ActivationFunctionType.Identity

    with TileContext(nc) as tc:
        with (
            tc.tile_pool(name="sb", bufs=1) as sb,
            tc.tile_pool(name="ps", bufs=1, space="PSUM") as psp,
        ):
            x_t = sb.tile([P, KC1 * XBLK + XSLACK], f16)
            # one tile per conv1 tap (both mc halves, 2560B HBM runs)
            w1_ts = [sb.tile([P, KC1 * MC * P], f16, tag=f"w1_{t}",
                             name=f"w1t{t}") for t in range(9)]
            w2_ts = [sb.tile([P, 3 * KC2 * MC * P], f16, tag=f"w2_{t}",
                             name=f"w2t{t}") for t in range(3)]
            wr_t = sb.tile([P, KC2 * OUT_C], f16)
            bm_t = sb.tile([2, BMW], f16)
            b2_t = sb.tile([P, MC], f32)
            h1_t = sb.tile([P, MC * H1BLK + H1SLACK], f16)
            h2_t = sb.tile([P, MC * H2BLK], f16)
            ro_t = sb.tile([W, R2 * OUT_C], f32)

            # ---- DMA schedule ------------------------------------------
            # Long per-partition HBM runs (>=2.5KB) are mandatory for queue
            # throughput, so granularity comes from PARTITION-splitting:
            # x loads as 3 partition-chunks, tap0 as 2 partition-halves.
            XW = KC1 * XBLK + XSLACK
            w1blk = KC1 * MC * P
            w2blk = 3 * KC2 * MC * P

            def dma_xp(eng, a, b):
                eng.dma_start(x_t[a:b, :],
                              bass.AP(xs_d, a * XW, [[XW, b - a], [1, XW]]))

            def dma_w1p(eng, t, a, b):
                eng.dma_start(w1_ts[t][a:b, :],
                              bass.AP(w1_d, t * w1blk + a * 9 * w1blk,
                                      [[9 * w1blk, b - a], [1, w1blk]]))

            def dma_w2(eng, t):
                eng.dma_start(w2_ts[t][:],
                              bass.AP(w2_d, t * w2blk,
                                      [[3 * w2blk, P], [1, w2blk]]))

            # HBM bandwidth (~300GB/s/core) is shared per in-flight transfer;
            # the gpsimd queue is strictly serial, sync/scalar keep ~3 in
            # flight.  So: gpsimd streams the ordered critical sequence; the
            # sync queue is force-serialized by padding each real transfer
            # with tiny scratch DMAs that exhaust its semaphore pool; scalar
            # (needed for ReLUs later) only carries late bulk.
            scr_ts = [sb.tile([1, 8], f16, tag=f"scr{i}", name=f"scr{i}")
                      for i in range(14)]
            tiny = iter(scr_ts)

            def dma_tiny(eng):
                eng.dma_start(next(tiny)[:], bass.AP(bm_d, 0, [[BMW, 1], [1, 8]]))

            # queue G (gpsimd, serial): x, odd taps, bm
            dma_xp(nc.gpsimd, 0, P)
            dma_w1p(nc.gpsimd, 1, 0, P)
            dma_w1p(nc.gpsimd, 3, 0, P)
            dma_w1p(nc.gpsimd, 5, 0, P)
            dma_w1p(nc.gpsimd, 7, 0, P)
            nc.gpsimd.dma_start(bm_t[:], bm_d.ap())
            # queue S (sync, serialized via scratch padding): even taps, w2_1
            dma_w1p(nc.sync, 0, 0, P)
            dma_tiny(nc.sync)
            dma_tiny(nc.sync)
            dma_w1p(nc.sync, 2, 0, P)
            dma_tiny(nc.sync)
            dma_tiny(nc.sync)
            dma_w1p(nc.sync, 4, 0, P)
            dma_tiny(nc.sync)
            dma_tiny(nc.sync)
            dma_w1p(nc.sync, 6, 0, P)
            dma_tiny(nc.sync)
            dma_tiny(nc.sync)
            dma_w1p(nc.sync, 8, 0, P)
            dma_w2(nc.sync, 1)
            # queue A (scalar): tiny front-padding keeps this ring drained
            # early (so it doesn't steal a DGE service slot from x/t0),
            # then the late bulk
            for _ in range(6):
                dma_tiny(nc.scalar)
            dma_w2(nc.scalar, 0)
            dma_w2(nc.scalar, 2)
            nc.scalar.dma_start(b2_t[:], b2_d.ap())
            nc.scalar.dma_start(wr_t[:], wr_d.ap())

            # h1 pads must be zero; activation only writes valid 32-col spans.
            nc.vector.memset(h1_t[:], 0.0)

            # PE warm-up: the tensor engine takes ~3us of continuous work to
            # reach max P-state; burn idle DMA-wait time on dummy matmuls so
            # conv1 runs at full clock from its first instruction.
            psw = psp.tile([P, 400], f32, tag="warm")
            for _ in range(14):
                nc.tensor.matmul(psw[:, :], lhsT=h1_t[:, 0:P],
                                 rhs=h1_t[:, 0:400], start=True, stop=True)

            # ---- conv1: 576 -> 256 over 6 rows --------------------------
            # both output-channel halves interleave per weight tap (separate
            # PSUM banks) so the PE keeps pace with the weight DMA stream
            ps1s = [psp.tile([P, N1], f32, tag="cva", name="ps1a"),
                    psp.tile([P, N1], f32, tag="cvb", name="ps1b")]
            for tap in range(9):
                ky, kx = tap // 3, tap % 3
                off = ky * WP + kx
                for kc in range(KC1):
                    for mc in range(MC):
                        nc.tensor.matmul(
                            ps1s[mc][:, :],
                            lhsT=w1_ts[tap][:, (kc * MC + mc) * P:
                                            (kc * MC + mc) * P + P],
                            rhs=x_t[:, kc * XBLK + off: kc * XBLK + off + N1],
                            start=(tap == 0 and kc == 0), stop=False,
                        )
            for mc in range(MC):
                # rank-2 update: psum += b1[mc] x inmask + (-1000) x outmask
                # (out-of-image halo rows pick up real x rows through the
                # shifted taps; -1000 drives them negative so ReLU yields 0)
                nc.tensor.matmul(
                    ps1s[mc][:, :],
                    lhsT=bm_t[0:2, 256 + mc * P: 256 + mc * P + P],
                    rhs=bm_t[0:2, 0:N1],
                    start=False, stop=True,
                )
            for mc in range(MC):
                # ReLU into the valid 32-wide spans of padded h1 rows
                ps1 = ps1s[mc]
                src = bass.AP(ps1.tensor, ps1.offset,
                              [[N1, P], [WP, R1], [1, 32]])
                dstb = h1_t[:, :]
                dst = bass.AP(dstb.tensor, dstb.offset + mc * H1BLK + 1,
                              [[MC * H1BLK + H1SLACK, P], [WP, R1], [1, 32]])
                nc.scalar.activation(dst, src, Relu)

            # ---- conv2: 256 -> 256 over 4 rows, mc-major ----------------
            # mc0's ReLU and 1x1 pass overlap mc1's accumulation chain
            ps2s = [psp.tile([P, N2], f32, tag="cvc", name="ps2a"),
                    psp.tile([P, N2], f32, tag="cvd", name="ps2b")]
            psr = psp.tile([W, R2 * OUT_C], f32, tag="psr")

            def conv2_chain(mc):
                for tap in range(9):
                    ky, kx = tap // 3, tap % 3
                    off = ky * WP + kx
                    for kc in range(KC2):
                        base = ((tap % 3) * KC2 + kc) * MC * P + mc * P
                        nc.tensor.matmul(
                            ps2s[mc][:, :],
                            lhsT=w2_ts[tap // 3][:, base: base + P],
                            rhs=h1_t[:, kc * H1BLK + off: kc * H1BLK + off + N2],
                            start=(tap == 0 and kc == 0),
                            stop=(tap == 8 and kc == KC2 - 1),
                        )

            def h2_relu(mc):
                ps2 = ps2s[mc]
                src2 = bass.AP(ps2.tensor, ps2.offset,
                               [[N2, P], [WP, R2], [1, 32]])
                h2b = h2_t[:, :]
                dst2 = bass.AP(h2b.tensor, h2b.offset + mc * H2BLK,
                               [[MC * H2BLK, P], [WP, R2], [1, 32]])
                nc.scalar.activation(dst2, src2, Relu, bias=b2_t[:, mc:mc + 1])

            conv2_chain(0)
            h2_relu(0)
            conv2_chain(1)
            h2_relu(1)
            # 1x1 conv 256 -> 24, transposed into (w, (h, c)); bias br is
            # added on the host during unsharding
            for h in range(R2):
                for kc in range(KC2):
                    nc.tensor.matmul(
                        psr[:, h * OUT_C:(h + 1) * OUT_C],
                        lhsT=h2_t[:, kc * H2BLK + h * WP: kc * H2BLK + h * WP + 32],
                        rhs=wr_t[:, kc * OUT_C:(kc + 1) * OUT_C],
                        start=(kc == 0), stop=(kc == KC2 - 1),
                    )
            nc.scalar.activation(ro_t[:, :], psr[:, :], Ident)
            nc.sync.dma_start(out_d.ap(), ro_t[:])

    nc.compile()
    return nc


def _pack_inputs(x, w1, b1, w2, b2, wr, br):
    x = np.asarray(x, np.float32)
    w1 = np.asarray(w1, np.float32)
    w2 = np.asarray(w2, np.float32)
    wr = np.asarray(wr, np.float32)
    b1 = np.asarray(b1, np.float32)
    b2 = np.asarray(b2, np.float32)

    xp = np.zeros((NCORES, P, KC1, RX, WP), np.float16)
    xv = x[0]  # (576, 32, 32)
    for k in range(NCORES):
        for r in range(RX):
            g = 4 * k - 2 + r
            if 0 <= g < H:
                blkv = xv[:, g, :]  # (576, 32)
                xp[k, :, :4, r, 1:33] = blkv[:512].reshape(4, P, W).transpose(1, 0, 2)
                xp[k, :64, 4, r, 1:33] = blkv[512:]
    xp = xp.reshape(NCORES, P, KC1 * XBLK)
    xp = np.concatenate([xp, np.zeros((NCORES, P, XSLACK), np.float16)], axis=2)

    # w1: [p, tap, kc, mc, m] = w1[mc*128+m, kc*128+p, ky, kx]
    w1p = np.zeros((P, 9, KC1, MC, P), np.float16)
    w1v = w1.transpose(2, 3, 1, 0).reshape(9, IN_C, MID_C)  # (tap, ci, co)
    w1p[:, :, :4, :, :] = (
        w1v[:, :512, :].reshape(9, 4, P, MC, P).transpose(2, 0, 1, 3, 4))
    w1p[:64, :, 4, :, :] = w1v[:, 512:, :].reshape(9, 64, MC, P).transpose(1, 0, 2, 3)
    w1p = w1p.reshape(P, 9 * KC1 * MC * P)

    w2p = np.zeros((P, 9, KC2, MC, P), np.float16)
    w2v = w2.transpose(2, 3, 1, 0).reshape(9, MID_C, MID_C)
    w2p[:, :, :, :, :] = (
        w2v.reshape(9, KC2, P, MC, P).transpose(2, 0, 1, 3, 4))
    w2p = w2p.reshape(P, 9 * KC2 * MC * P)

    wrp = wr.T.reshape(KC2, P, OUT_C).transpose(1, 0, 2).reshape(P, KC2 * OUT_C)
    wrp = np.ascontiguousarray(wrp, np.float16)
    b2p = b2.reshape(MC, P).T.copy()

    # biasmask rows per core: row0 = [0:N1] in-image mask, [256:512] b1;
    # row1 = [0:N1] out-of-image mask, [256:512] -1000 (ReLU clamp)
    bmp = np.zeros((NCORES, 2, BMW), np.float16)
    for k in range(NCORES):
        for r in range(R1):
            row = 0 if 0 <= 4 * k - 1 + r < H else 1
            bmp[k, row, r * WP:r * WP + WP] = 1.0
        bmp[k, 0, 256:384] = b1[:P].astype(np.float16)
        bmp[k, 0, 384:512] = b1[P:].astype(np.float16)
        bmp[k, 1, 256:512] = -1000.0

    shared = dict(w1p=w1p, w2p=w2p, wrp=wrp, b2p=b2p)
    in_maps = []
    for k in range(NCORES):
        m = dict(shared)
        m["xs"] = np.ascontiguousarray(xp[k])
        m["bmp"] = np.ascontiguousarray(bmp[k])
        in_maps.append(m)
    return in_maps


def kernel(x, w1, b1, w2, b2, wr, br):
    from concourse.bass_utils import run_bass_kernel_spmd

    if "nc" not in _prog_cache:
        _prog_cache["nc"] = _build_program()
    nc = _prog_cache["nc"]

    in_maps = _pack_inputs(x, w1, b1, w2, b2, wr, br)
    res = run_bass_kernel_spmd(nc, in_maps, list(range(NCORES)))

    # unshard: gather per-core reduced rows, then exact fp32 pool expansion
    r = np.empty((OUT_C, H, W), np.float32)
    for k in range(NCORES):
        buf = np.asarray(res.results[k]["outb"], np.float32)  # (32, 96)
        r[:, 4 * k:4 * k + 4, :] = buf.reshape(W, R2, OUT_C).transpose(2, 1, 0)
    r += np.asarray(br, np.float32)[:, None, None]
    rw = np.repeat(r, OUT_W // W, axis=2)          # (24, 32, 1024)
    i = np.arange(OUT_H)
    s = (i * H) // OUT_H
    e = -((-(i + 1) * H) // OUT_H)
    ln = (e - s).astype(np.float32)[None, :, None]
    out = (rw[:, s, :] + rw[:, e - 1, :] * (ln - 1.0)) / ln
    return np.ascontiguousarray(out[None], np.float32)


# revision 29
# speedup vs baseline: 1.0813x; 1.0813x over previous
"""Trainium2 Bass kernel for nn_ConvProjector (conv3x3 -> ReLU -> conv3x3 -> ReLU
-> adaptive-avg-pool upsample 32x32 -> 687x1024 -> 1x1 conv 256->24 + bias).

Strategy:
  * The adaptive pool (linear) and the 1x1 conv (linear) commute: apply the
    256->24 channel reduction at 32x32 resolution first.  The device then
    emits only the DISTINCT values of the output: the reduced tensor
    r = wr @ h2 at 32x32 (every output column is a pure replica of an input
    column since 1024 = 32*32, and every output row is either a replica or
    the mean of two adjacent rows).  The pool expansion to 687x1024 plus the
    +bias is exact fp32 gather arithmetic done while unsharding on the host.
  * Sharding: 8 cores, core k owns rows 4k..4k+3.  It computes h1 rows
    4k-1..4k+4 (conv halos) from x rows 4k-2..4k+5, h2/r rows 4k..4k+3,
    channel-complete.  No collectives.
  * conv1 bias+image-row masking is folded into the matmul accumulation
    (rank-1 update b1 x maskrow), so no mask tensor or vector ops.
  * All matmuls run in fp16 (inputs pre-rounded on host; PSUM stays fp32).
  * Weight DMA: per-(tap, mc) w1 tiles and per-kc x chunks are assigned to
    the three DMA-capable queues (sync/scalar/gpsimd) by a greedy scheduler
    in consumption order, so conv1 starts early and streams behind arrival.
"""
import sys

if '/opt/trn_rl_repo' not in sys.path:
    sys.path.insert(0, '/opt/trn_rl_repo')

import numpy as np

IN_C, MID_C, OUT_C = 576, 256, 24
H = W = 32
OUT_H, OUT_W = 687, 1024
NCORES = 8
P = 128
KC1 = 5           # ceil(576/128) input-channel chunks for conv1 (padded to 640)
KC2 = 2           # 256/128 chunks for conv2 / 1x1
MC = 2            # 256/128 output-channel chunks for conv1/conv2
WP = 34           # padded row width (1 zero col each side)
RX, R1, R2 = 8, 6, 4          # x rows / h1 rows / h2 (=r) rows per core
XBLK = RX * WP                # 272  per-kc x block
XSLACK = 16                   # rhs overrun slack
N1 = R1 * WP                  # 204  conv1 matmul N
H1BLK = R1 * WP               # 204  per-mc h1 block
H1SLACK = 16
N2 = R2 * WP                  # 136  conv2 matmul N
H2BLK = R2 * WP               # 136  h2 flat span per mc
BMW = 512                     # biasmask row: [0:N1] mask, [256:384] b1 mc0,
                              # [384:512] b1 mc1

_prog_cache = {}


def _build_program():
    import concourse.bass as bass
    import concourse.bacc as bacc
    import concourse.mybir as mybir
    from concourse.tile import TileContext

    f32 = mybir.dt.float32
    f16 = mybir.dt.float16
    nc = bacc.Bacc("TRN2", target_bir_lowering=False, debug=False,
                   num_devices=NCORES)

    xs_d = nc.dram_tensor("xs", [P, KC1 * XBLK + XSLACK], f16, kind="ExternalInput")
    w1_d = nc.dram_tensor("w1p", [P, 9 * KC1 * MC * P], f16, kind="ExternalInput")
    w2_d = nc.dram_tensor("w2p", [P, 9 * KC2 * MC * P], f16, kind="ExternalInput")
    wr_d = nc.dram_tensor("wrp", [P, KC2 * OUT_C], f16, kind="ExternalInput")
    bm_d = nc.dram_tensor("bmp", [2, BMW], f16, kind="ExternalInput")
    b2_d = nc.dram_tensor("b2p", [P, MC], f32, kind="ExternalInput")
    out_d = nc.dram_tensor("outb", [W, R2 * OUT_C], f32, kind="ExternalOutput")

    Relu = mybir.ActivationFunctionType.Relu
    Ident = mybir.ActivationFunctionType.Identity

    with TileContext(nc) as tc:
        with (
            tc.tile_pool(name="sb", bufs=1) as sb,
            tc.tile_pool(name="ps", bufs=1, space="PSUM") as psp,
        ):
            x_t = sb.tile([P, KC1 * XBLK + XSLACK], f16)
            # one tile per conv1 tap (both mc halves, 2560B HBM runs)
            w1_ts = [sb.tile([P, KC1 * MC * P], f16, tag=f"w1_{t}",
                             name=f"w1t{t}") for t in range(9)]
            w2_ts = [sb.tile([P, 3 * KC2 * MC * P], f16, tag=f"w2_{t}",
                             name=f"w2t{t}") for t in range(3)]
            wr_t = sb.tile([P, KC2 * OUT_C], f16)
            bm_t = sb.tile([2, BMW], f16)
            b2_t = sb.tile([P, MC], f32)
            h1_t = sb.tile([P, MC * H1BLK + H1SLACK], f16)
            h2_t = sb.tile([P, MC * H2BLK], f16)
            ro_t = sb.tile([W, R2 * OUT_C], f32)

            # ---- DMA schedule ------------------------------------------
            # Long per-partition HBM runs (>=2.5KB) are mandatory for queue
            # throughput, so granularity comes from PARTITION-splitting:
            # x loads as 3 partition-chunks, tap0 as 2 partition-halves.
            XW = KC1 * XBLK + XSLACK
            w1blk = KC1 * MC * P
            w2blk = 3 * KC2 * MC * P

            def dma_xp(eng, a, b):
                eng.dma_start(x_t[a:b, :],
                              bass.AP(xs_d, a * XW, [[XW, b - a], [1, XW]]))

            def dma_w1p(eng, t, a, b):
                eng.dma_start(w1_ts[t][a:b, :],
                              bass.AP(w1_d, t * w1blk + a * 9 * w1blk,
                                      [[9 * w1blk, b - a], [1, w1blk]]))

            def dma_w2(eng, t):
                eng.dma_start(w2_ts[t][:],
                              bass.AP(w2_d, t * w2blk,
                                      [[3 * w2blk, P], [1, w2blk]]))

            # HBM bandwidth (~300GB/s/core) is shared per in-flight transfer;
            # the gpsimd queue is strictly serial, sync/scalar keep ~3 in
            # flight.  So: gpsimd streams the ordered critical sequence; the
            # sync queue is force-serialized by padding each real transfer
            # with tiny scratch DMAs that exhaust its semaphore pool; scalar
            # (needed for ReLUs later) only carries late bulk.
            scr_ts = [sb.tile([1, 8], f16, tag=f"scr{i}", name=f"scr{i}")
                      for i in range(14)]
            tiny = iter(scr_ts)

            def dma_tiny(eng):
                eng.dma_start(next(tiny)[:], bass.AP(bm_d, 0, [[BMW, 1], [1, 8]]))

            # queue G (gpsimd, serial): bm, odd taps, then w2 bulk
            nc.gpsimd.dma_start(bm_t[:], bm_d.ap())
            dma_w1p(nc.gpsimd, 1, 0, P)
            dma_w1p(nc.gpsimd, 3, 0, P)
            dma_w1p(nc.gpsimd, 5, 0, P)
            dma_w1p(nc.gpsimd, 7, 0, P)
            dma_w2(nc.gpsimd, 0)
            dma_w2(nc.gpsimd, 2)
            # queue S (sync, serialized via scratch padding): even taps, w2_1
            dma_w1p(nc.sync, 0, 0, P)
            dma_tiny(nc.sync)
            dma_tiny(nc.sync)
            dma_w1p(nc.sync, 2, 0, P)
            dma_tiny(nc.sync)
            dma_tiny(nc.sync)
            dma_w1p(nc.sync, 4, 0, P)
            dma_tiny(nc.sync)
            dma_tiny(nc.sync)
            dma_w1p(nc.sync, 6, 0, P)
            dma_tiny(nc.sync)
            dma_tiny(nc.sync)
            dma_w1p(nc.sync, 8, 0, P)
            dma_w2(nc.sync, 1)
            # queue A (scalar): x first (this ring is serviced immediately),
            # then only small late tensors so ReLUs aren't blocked
            dma_xp(nc.scalar, 0, P)
            nc.scalar.dma_start(b2_t[:], b2_d.ap())
            nc.scalar.dma_start(wr_t[:], wr_d.ap())

            # h1 pads must be zero; activation only writes valid 32-col spans.
            nc.vector.memset(h1_t[:], 0.0)

            # PE warm-up: the tensor engine takes ~3us of continuous work to
            # reach max P-state; burn idle DMA-wait time on dummy matmuls so
            # conv1 runs at full clock from its first instruction.
            psw = psp.tile([P, 400], f32, tag="warm")
            for _ in range(14):
                nc.tensor.matmul(psw[:, :], lhsT=h1_t[:, 0:P],
                                 rhs=h1_t[:, 0:400], start=True, stop=True)

            # ---- conv1: 576 -> 256 over 6 rows --------------------------
            # both output-channel halves interleave per weight tap (separate
            # PSUM banks) so the PE keeps pace with the weight DMA stream
            ps1s = [psp.tile([P, N1], f32, tag="cva", name="ps1a"),
                    psp.tile([P, N1], f32, tag="cvb", name="ps1b")]
            for tap in range(9):
                ky, kx = tap // 3, tap % 3
                off = ky * WP + kx
                for kc in range(KC1):
                    for mc in range(MC):
                        nc.tensor.matmul(
                            ps1s[mc][:, :],
                            lhsT=w1_ts[tap][:, (kc * MC + mc) * P:
                                            (kc * MC + mc) * P + P],
                            rhs=x_t[:, kc * XBLK + off: kc * XBLK + off + N1],
                            start=(tap == 0 and kc == 0), stop=False,
                        )
            for mc in range(MC):
                # rank-2 update: psum += b1[mc] x inmask + (-1000) x outmask
                # (out-of-image halo rows pick up real x rows through the
                # shifted taps; -1000 drives them negative so ReLU yields 0)
                nc.tensor.matmul(
                    ps1s[mc][:, :],
                    lhsT=bm_t[0:2, 256 + mc * P: 256 + mc * P + P],
                    rhs=bm_t[0:2, 0:N1],
                    start=False, stop=True,
                )
            for mc in range(MC):
                # ReLU into the valid 32-wide spans of padded h1 rows
                ps1 = ps1s[mc]
                src = bass.AP(ps1.tensor, ps1.offset,
                              [[N1, P], [WP, R1], [1, 32]])
                dstb = h1_t[:, :]
                dst = bass.AP(dstb.tensor, dstb.offset + mc * H1BLK + 1,
                              [[MC * H1BLK + H1SLACK, P], [WP, R1], [1, 32]])
                nc.scalar.activation(dst, src, Relu)

            # ---- conv2: 256 -> 256 over 4 rows, mc-major ----------------
            # mc0's ReLU and 1x1 pass overlap mc1's accumulation chain
            ps2s = [psp.tile([P, N2], f32, tag="cvc", name="ps2a"),
                    psp.tile([P, N2], f32, tag="cvd", name="ps2b")]
            psr = psp.tile([W, R2 * OUT_C], f32, tag="psr")

            def conv2_chain(mc):
                for tap in range(9):
                    ky, kx = tap // 3, tap % 3
                    off = ky * WP + kx
                    for kc in range(KC2):
                        base = ((tap % 3) * KC2 + kc) * MC * P + mc * P
                        nc.tensor.matmul(
                            ps2s[mc][:, :],
                            lhsT=w2_ts[tap // 3][:, base: base + P],
                            rhs=h1_t[:, kc * H1BLK + off: kc * H1BLK + off + N2],
                            start=(tap == 0 and kc == 0),
                            stop=(tap == 8 and kc == KC2 - 1),
                        )

            def h2_relu(mc):
                ps2 = ps2s[mc]
                src2 = bass.AP(ps2.tensor, ps2.offset,
                               [[N2, P], [WP, R2], [1, 32]])
                h2b = h2_t[:, :]
                dst2 = bass.AP(h2b.tensor, h2b.offset + mc * H2BLK,
                               [[MC * H2BLK, P], [WP, R2], [1, 32]])
                nc.scalar.activation(dst2, src2, Relu, bias=b2_t[:, mc:mc + 1])

            conv2_chain(0)
            h2_relu(0)
            conv2_chain(1)
            h2_relu(1)
            # 1x1 conv 256 -> 24, transposed into (w, (h, c)); bias br is
            # added on the host during unsharding
            for h in range(R2):
                for kc in range(KC2):
                    nc.tensor.matmul(
                        psr[:, h * OUT_C:(h + 1) * OUT_C],
                        lhsT=h2_t[:, kc * H2BLK + h * WP: kc * H2BLK + h * WP + 32],
                        rhs=wr_t[:, kc * OUT_C:(kc + 1) * OUT_C],
                        start=(kc == 0), stop=(kc == KC2 - 1),
                    )
            nc.scalar.activation(ro_t[:, :], psr[:, :], Ident)
            nc.sync.dma_start(out_d.ap(), ro_t[:])

    nc.compile()
    return nc


def _pack_inputs(x, w1, b1, w2, b2, wr, br):
    x = np.asarray(x, np.float32)
    w1 = np.asarray(w1, np.float32)
    w2 = np.asarray(w2, np.float32)
    wr = np.asarray(wr, np.float32)
    b1 = np.asarray(b1, np.float32)
    b2 = np.asarray(b2, np.float32)

    xp = np.zeros((NCORES, P, KC1, RX, WP), np.float16)
    xv = x[0]  # (576, 32, 32)
    for k in range(NCORES):
        for r in range(RX):
            g = 4 * k - 2 + r
            if 0 <= g < H:
                blkv = xv[:, g, :]  # (576, 32)
                xp[k, :, :4, r, 1:33] = blkv[:512].reshape(4, P, W).transpose(1, 0, 2)
                xp[k, :64, 4, r, 1:33] = blkv[512:]
    xp = xp.reshape(NCORES, P, KC1 * XBLK)
    xp = np.concatenate([xp, np.zeros((NCORES, P, XSLACK), np.float16)], axis=2)

    # w1: [p, tap, kc, mc, m] = w1[mc*128+m, kc*128+p, ky, kx]
    w1p = np.zeros((P, 9, KC1, MC, P), np.float16)
    w1v = w1.transpose(2, 3, 1, 0).reshape(9, IN_C, MID_C)  # (tap, ci, co)
    w1p[:, :, :4, :, :] = (
        w1v[:, :512, :].reshape(9, 4, P, MC, P).transpose(2, 0, 1, 3, 4))
    w1p[:64, :, 4, :, :] = w1v[:, 512:, :].reshape(9, 64, MC, P).transpose(1, 0, 2, 3)
    w1p = w1p.reshape(P, 9 * KC1 * MC * P)

    w2p = np.zeros((P, 9, KC2, MC, P), np.float16)
    w2v = w2.transpose(2, 3, 1, 0).reshape(9, MID_C, MID_C)
    w2p[:, :, :, :, :] = (
        w2v.reshape(9, KC2, P, MC, P).transpose(2, 0, 1, 3, 4))
    w2p = w2p.reshape(P, 9 * KC2 * MC * P)

    wrp = wr.T.reshape(KC2, P, OUT_C).transpose(1, 0, 2).reshape(P, KC2 * OUT_C)
    wrp = np.ascontiguousarray(wrp, np.float16)
    b2p = b2.reshape(MC, P).T.copy()

    # biasmask rows per core: row0 = [0:N1] in-image mask, [256:512] b1;
    # row1 = [0:N1] out-of-image mask, [256:512] -1000 (ReLU clamp)
    bmp = np.zeros((NCORES, 2, BMW), np.float16)
    for k in range(NCORES):
        for r in range(R1):
            row = 0 if 0 <= 4 * k - 1 + r < H else 1
            bmp[k, row, r * WP:r * WP + WP] = 1.0
        bmp[k, 0, 256:384] = b1[:P].astype(np.float16)
        bmp[k, 0, 384:512] = b1[P:].astype(np.float16)
        bmp[k, 1, 256:512] = -1000.0

    shared = dict(w1p=w1p, w2p=w2p, wrp=wrp, b2p=b2p)
    in_maps = []
    for k in range(NCORES):
        m = dict(shared)
        m["xs"] = np.ascontiguousarray(xp[k])
        m["bmp"] = np.ascontiguousarray(bmp[k])
        in_maps.append(m)
    return in_maps


def kernel(x, w1, b1, w2, b2, wr, br):
    from concourse.bass_utils import run_bass_kernel_spmd

    if "nc" not in _prog_cache:
        _prog_cache["nc"] = _build_program()
    nc = _prog_cache["nc"]

    in_maps = _pack_inputs(x, w1, b1, w2, b2, wr, br)
    res = run_bass_kernel_spmd(nc, in_maps, list(range(NCORES)))

    # unshard: gather per-core reduced rows, then exact fp32 pool expansion
    r = np.empty((OUT_C, H, W), np.float32)
    for k in range(NCORES):
        buf = np.asarray(res.results[k]["outb"], np.float32)  # (32, 96)
        r[:, 4 * k:4 * k + 4, :] = buf.reshape(W, R2, OUT_C).transpose(2, 1, 0)
    r += np.asarray(br, np.float32)[:, None, None]
    rw = np.repeat(r, OUT_W // W, axis=2)          # (24, 32, 1024)
    i = np.arange(OUT_H)
    s = (i * H) // OUT_H
    e = -((-(i + 1) * H) // OUT_H)
    ln = (e - s).astype(np.float32)[None, :, None]
    out = (rw[:, s, :] + rw[:, e - 1, :] * (ln - 1.0)) / ln
    return np.ascontiguousarray(out[None], np.float32)
